# revision 5
# baseline (speedup 1.0000x reference)
"""Trainium2 Bass kernel for nn_Compressor (sparse_attention, hierarchical window MLP).

Reference computation (per batch b, head h):
  windows w=0..510 over k[b,h] (S=8192, D=128), window length 32, stride 16
  x[w, l, :] = k[16w+l, :] + pe[l, :]
  5 stages of pairwise-merge MLP: x <- silu(x.reshape(-1, 256) @ w_down[i].T)
  out[w+1] = x @ w_stop.T   ; out[0] = 0 (prepended zero window)

Sharding: head-parallel across 8 cores (B*H = 32 -> 4 heads/core), weights
replicated, no cross-device comms.

Algebraic optimization: stage-0 operates on adjacent row pairs (s=2t, 2t+1)
and every pair is shared by exactly two windows (stride 16, pair width 2),
always in the same even/odd role.  So
  Z[:, t] = W0_even @ kT[:, 2t] + W0_odd @ kT[:, 2t+1]
is computed once per pair (half the naive stage-0 flops).  The host pre-adds
pe[l] (the use-A positional encoding, position l = s mod 16) into k itself,
so the use-A silu needs no bias; the second use of each pair (use B,
position l+16 in the previous window) differs only by the linear image of
the pe difference,
  dpe0 = W0 @ (pe_B - pe_A)  [128, 8],
which a K=1 PE matmul (dpe column x ones-row outer product) accumulates into
the Z PSUM bank between the two silu reads.  Both stage-0 silus are then
bias-free, so each ScalarE ACTIVATE spans a 2-plane PSUM group (N=1022),
halving the ACT instruction count of the dominant phase (ACT is the
bottleneck engine: every silu element costs 1 lane-cycle at 1.2 GHz).

Layout: everything is kept "plane-major" so every matmul moving operand,
every activation input/output, and every copy is contiguous:
  ktp[d, l, w]   = bf16 (k[16w + l, d] + pe[l, d])  -- host provides this
                   fully transposed, so the HBM->SBUF DMA is a straight
                   contiguous copy (no xbar transpose on the device)
  Z group g      = psum [128, 2, 512]: plane e=2g+i from W0e/W0o matmuls
  s{i}[d, p, w]  = silu-merged planes, stage i
The final w_stop matmul uses the data as the stationary operand, producing
output already row-major for a clean single DMA out per head.
"""

import numpy as np

B, H, S, D = 2, 16, 8192, 128
BH = B * H
NCORES = 8
HPC = BH // NCORES  # heads per core = 4
NB = (S - 32) // 16 + 1  # 511 sliding windows
NW = NB + 1  # 512 output rows per head (incl. zero window)

# w_stop output chunking: window ranges per PE (stationary) chunk
QRANGES = [(0, 128), (128, 128), (256, 128), (384, 127)]

_BASS_CACHE = {}


def _build_bass():
    import concourse.bacc as bacc
    import concourse.mybir as mybir
    import concourse.tile as tile
    from bass_rust import add_dep_helper

    f32 = mybir.dt.float32
    bf16 = mybir.dt.bfloat16
    SILU = mybir.ActivationFunctionType.Silu

    nc = bacc.Bacc()
    # k4p[hh, d, 512*l + w] = bf16(k[16w + l, d] + pe[l, d]): the l-planar
    # transposed layout, prepared on the host so the device DMA is contiguous.
    k4p = nc.dram_tensor("k4p", [HPC, D, S], bf16, kind="ExternalInput")
    # wdt[k, i, h, o] = w_down[i][o, 128h + k]: already in SBUF layout
    wdt = nc.dram_tensor("wdt", [128, 5, 2, 128], bf16, kind="ExternalInput")
    dpeT = nc.dram_tensor("dpeT", [1, 8, 128], bf16, kind="ExternalInput")
    ones = nc.dram_tensor("ones", [1, 512], bf16, kind="ExternalInput")
    wst = nc.dram_tensor("wst", [128, 128], bf16, kind="ExternalInput")
    oqs = [
        nc.dram_tensor(f"o{hh}", [513, 128], f32, kind="ExternalOutput")
        for hh in range(HPC)
    ]

    with tile.TileContext(nc) as tc:
        with (
            tc.tile_pool(name="consts", bufs=1) as consts,
            tc.tile_pool(name="ktp", bufs=2) as ktp,
            tc.tile_pool(name="s0p", bufs=2) as s0p,
            tc.tile_pool(name="stp", bufs=2) as stp,
            tc.tile_pool(name="outp", bufs=2) as outp,
            tc.tile_pool(name="tps", bufs=1, space="PSUM") as tps,
            tc.tile_pool(name="zps", bufs=2, space="PSUM") as zps,
            tc.tile_pool(name="sps", bufs=1, space="PSUM") as sps,
            tc.tile_pool(name="ops", bufs=1, space="PSUM") as ops,
        ):
            # constants go on the ACT HWDGE ring (separate queue from the k
            # loads on the SP ring), as plain contiguous copies
            wd_sb = consts.tile([128, 5, 2, 128], bf16, name="wd_sb")
            nc.scalar.dma_start(out=wd_sb, in_=wdt[:])
            wst_sb = consts.tile([128, 128], bf16, name="wst_sb")
            nc.scalar.dma_start(out=wst_sb, in_=wst[:])
            dpeT_sb = consts.tile([1, 8, 128], bf16, name="dpeT_sb")
            nc.scalar.dma_start(out=dpeT_sb, in_=dpeT[:])
            ones_sb = consts.tile([1, 512], bf16, name="ones_sb")
            nc.scalar.dma_start(out=ones_sb, in_=ones[:])

            # The walrus pipeline fuses a matmul's sem waits into its
            # LDWEIGHTS slot (1 wait); extra waits cost an EventSemaphore
            # instruction.  Absorber matmuls make PE observe each semaphore
            # cheaply first.  They write disjoint 2-col regions of one
            # never-read PSUM bank (no WAW sems between them).
            dummy = tps.tile([128, 512], f32, name="dummy", tag="dummy", bufs=1)
            dummy_ctr = [0]

            def absorb(lhsT, rhs):
                m = dummy_ctr[0]
                dummy_ctr[0] += 1
                dst = dummy[: lhsT.shape[-1], 2 * m : 2 * m + 2]
                return nc.tensor.matmul(dst, lhsT=lhsT, rhs=rhs, start=True, stop=True)

            absorb(wd_sb[:, 0, 0, 0:2], wd_sb[:, 0, 0, 0:2])
            absorb(wst_sb[:, 0:2], wst_sb[:, 0:2])
            absorb(dpeT_sb[:, 0, 0:2], ones_sb[:, 0:2])

            def z_units(hh, s0_out):
                """Generator: per-group units of the shared stage-0 (Z) phase.

                Group g covers Z planes e=2g, 2g+1 in one [128, 2, 512] PSUM
                tile (2 banks).  Sequence per group: 4 Z matmuls; silu A over
                cols 0:511 of both planes (s0 planes 2g, 2g+1); dpe bias
                outer-product matmuls into cols 1:512; silu B over cols 1:512
                (s0 planes 2g+8, 2g+9).  The bias matmuls of group g are
                issued after group g+1's Z matmuls so PE never waits on the
                silu-A WAR dependency.
                """
                ktf = ktp.tile([128, S], bf16, name="ktf")
                kt3 = ktf.rearrange("p (l w) -> p l w", w=512)
                # 4 chunked contiguous DMAs (4 l-planes each) so the first Z
                # matmuls start as soon as the first chunk lands
                for c in range(4):
                    nc.sync.dma_start(
                        out=ktf[:, 2048 * c : 2048 * (c + 1)],
                        in_=k4p[hh, :, 2048 * c : 2048 * (c + 1)],
                    )
                s0 = s0p.tile([128, 16, NB], bf16, name="s0")
                s0_out[hh] = s0

                def zmm_group(g):
                    # observe this chunk's DMA lane on PE
                    absorb(kt3[:, 4 * g, 0:2], kt3[:, 4 * g, 0:2])
                    zp = zps.tile([128, 2, 512], f32, name="zp", tag="zp")
                    for i in range(2):
                        e = 2 * g + i
                        nc.tensor.matmul(
                            zp[:, i, :], lhsT=wd_sb[:, 0, 0, :],
                            rhs=kt3[:, 2 * e, :], start=True, stop=False,
                        )
                        nc.tensor.matmul(
                            zp[:, i, :], lhsT=wd_sb[:, 0, 1, :],
                            rhs=kt3[:, 2 * e + 1, :], start=False, stop=True,
                        )
                    return zp

                def finish_group(g, zp):
                    for i in range(2):
                        e = 2 * g + i
                        nc.tensor.matmul(
                            zp[:, i, 1:512], lhsT=dpeT_sb[:, e, :],
                            rhs=ones_sb[:, 0:NB], start=False, stop=True,
                            skip_group_check=True,
                        )
                    nc.scalar.activation(
                        out=s0[:, 2 * g + 8 : 2 * g + 10, :],
                        in_=zp[:, :, 1 : NB + 1], func=SILU,
                    )

                prev = None
                for g in range(4):
                    zp = zmm_group(g)
                    if prev is not None:
                        finish_group(g - 1, prev)
                    nc.scalar.activation(
                        out=s0[:, 2 * g : 2 * g + 2, :],
                        in_=zp[:, :, 0:NB], func=SILU,
                    )
                    prev = zp
                    yield
                finish_group(3, prev)
                yield

            def stage_units(hh, s0):
                """Generator: per-silu-group units of stages 1..4 + w_stop."""
                prev = s0
                for st in range(1, 5):
                    nj = 16 >> st
                    cur = stp.tile([128, nj, NB], bf16, name=f"s{st}", tag=f"s{st}")
                    for p in range((nj + 1) // 2):
                        npl = min(2, nj - 2 * p)
                        ps = sps.tile([128, 2, 512], f32, name="ps", tag="sp")
                        for ii in range(npl):
                            i = 2 * p + ii
                            nc.tensor.matmul(
                                ps[:, ii, :NB], lhsT=wd_sb[:, st, 0, :],
                                rhs=prev[:, 2 * i, :],
                                start=True, stop=False,
                            )
                            nc.tensor.matmul(
                                ps[:, ii, :NB], lhsT=wd_sb[:, st, 1, :],
                                rhs=prev[:, 2 * i + 1, :],
                                start=False, stop=True,
                            )
                        nc.scalar.activation(
                            out=cur[:, 2 * p : 2 * p + npl, :],
                            in_=ps[:, :npl, :NB], func=SILU,
                        )
                        yield
                    prev = cur

                # w_stop with data-stationary -> row-major [w, o] output;
                # all 4 chunks packed into one PSUM bank, single DVE copy,
                # single DMA out (rows 1..512 of o[hh]; row 512 is garbage
                # from the 127-wide last chunk, sliced off on the host)
                s4f = prev[:, 0, :]  # [128, 511]
                outsb = outp.tile([128, 4, 128], f32, name="outsb")
                ab2 = absorb(s4f[:, 0:2], s4f[:, 0:2])
                ps2 = ops.tile([128, 4, 128], f32, name="ps2", tag="op")
                for q, (w0, wq) in enumerate(QRANGES):
                    mmq = nc.tensor.matmul(
                        ps2[:wq, q, :],
                        lhsT=s4f[:, w0 : w0 + wq],
                        rhs=wst_sb,
                        start=True, stop=True,
                    )
                    if q == 0:
                        add_dep_helper(mmq.ins, ab2.ins, False,
                                       "absorber before first stop matmul")
                nc.vector.tensor_copy(out=outsb, in_=ps2)
                o_view = oqs[hh][1:513].rearrange("(q p) o -> p q o", q=4)
                if hh == HPC - 1:
                    # SP ring is drained of k loads by now; HWDGE is faster
                    nc.sync.dma_start(out=o_view, in_=outsb)
                else:
                    nc.gpsimd.dma_start(out=o_view, in_=outsb)
                yield

            # software pipeline with fine-grained interleave: the stage/stop
            # units of head h alternate with the Z units of head h+1, so the
            # in-order ACT/PE streams always have independent work to backfill
            # dependency gaps
            s0s = {}
            for _ in z_units(0, s0s):
                pass
            for hh in range(HPC):
                gens = [stage_units(hh, s0s.pop(hh))]
                if hh + 1 < HPC:
                    gens.append(z_units(hh + 1, s0s))
                while gens:
                    for g in list(gens):
                        try:
                            next(g)
                        except StopIteration:
                            gens.remove(g)

    if not nc.is_finalized():
        nc.finalize()
    return nc


def _prep_host_inputs(k, pe, w_down, w_stop):
    import ml_dtypes

    bf16 = ml_dtypes.bfloat16
    k = np.asarray(k, dtype=np.float32)
    pe = np.asarray(pe, dtype=np.float32)
    w_down = np.asarray(w_down, dtype=np.float32)
    w_stop = np.asarray(w_stop, dtype=np.float32)

    # k4p[bh, d, 512*l + w] = k[bh, 16w+l, d] + pe[l, d], cast to bf16 (RNE):
    # the fully transposed l-planar layout with the use-A pe pre-added, so the
    # device-side DMA is a straight contiguous copy
    kr = k.reshape(BH, 512, 16, D) + pe[:16][None, None, :, :]
    k4p = np.ascontiguousarray(kr.transpose(0, 3, 2, 1)).astype(bf16)
    # wdt[k, i, h, o] = w_down[i][o, 128h + k]
    wdt = np.ascontiguousarray(
        w_down.reshape(5, 128, 2, 128).transpose(3, 0, 2, 1)
    ).astype(bf16)
    # dpeT[e, o] = (W0 @ (pe_pair(e+8) - pe_pair(e)))[o]: use-B bias correction
    pe_pairs = pe.reshape(16, 256).astype(np.float64)
    dpeT = (
        (pe_pairs[8:] - pe_pairs[:8]) @ w_down[0].astype(np.float64).T
    ).astype(np.float32).astype(bf16).reshape(1, 8, 128)
    ones = np.ones((1, 512), dtype=bf16)
    wst = np.ascontiguousarray(w_stop.T).astype(bf16)
    return k4p, wdt, dpeT, ones, wst


def run(k, pe, w_down, w_stop, trace=False, trace_kwargs=None):
    from concourse.bass_utils import run_bass_kernel_spmd

    k4p, wdt, dpeT, ones, wst = _prep_host_inputs(k, pe, w_down, w_stop)

    if "nc" not in _BASS_CACHE:
        _BASS_CACHE["nc"] = _build_bass()
    nc = _BASS_CACHE["nc"]

    in_maps = [
        {
            "k4p": np.ascontiguousarray(k4p[HPC * c : HPC * (c + 1)]),
            "wdt": wdt,
            "dpeT": dpeT,
            "ones": ones,
            "wst": wst,
        }
        for c in range(NCORES)
    ]
    res = run_bass_kernel_spmd(
        nc, in_maps, core_ids=list(range(NCORES)), trace=trace,
        **(trace_kwargs or {}),
    )
    out = np.empty((BH, NW, D), dtype=np.float32)
    for c in range(NCORES):
        r = res.results[c]
        for hh in range(HPC):
            row = HPC * c + hh
            out[row, 0, :] = 0.0
            out[row, 1:NW, :] = r[f"o{hh}"][1:NW]
    out = out.reshape(B, H, NW, D)
    return out, res


def kernel(k, pe, w_down, w_stop):
    out, _ = run(k, pe, w_down, w_stop, trace=False)
    return out


# revision 8
# speedup vs baseline: 1.1123x; 1.1123x over previous
"""Trainium2 Bass kernel for nn_Compressor (sparse_attention, hierarchical window MLP).

Reference computation (per batch b, head h):
  windows w=0..510 over k[b,h] (S=8192, D=128), window length 32, stride 16
  x[w, l, :] = k[16w+l, :] + pe[l, :]
  5 stages of pairwise-merge MLP: x <- silu(x.reshape(-1, 256) @ w_down[i].T)
  out[w+1] = x @ w_stop.T   ; out[0] = 0 (prepended zero window)

Sharding: head-parallel across 8 cores (B*H = 32 -> 4 heads/core), weights
replicated, no cross-device comms.

Algebraic optimization: stage-0 operates on adjacent row pairs (s=2t, 2t+1)
and every pair is shared by exactly two windows (stride 16, pair width 2),
always in the same even/odd role.  So
  Z[:, t] = W0_even @ kT[:, 2t] + W0_odd @ kT[:, 2t+1]
is computed once per pair (half the naive stage-0 flops).  The host pre-adds
pe[l] (the use-A positional encoding, position l = s mod 16) into k itself,
so the use-A silu needs no bias; the second use of each pair (use B,
position l+16 in the previous window) differs only by the linear image of
the pe difference,
  dpe0 = W0 @ (pe_B - pe_A)  [128, 8],
which a K=1 PE matmul (dpe column x ones-row outer product) accumulates into
the Z PSUM bank between the two silu reads.  Both stage-0 silus are then
bias-free, so each ScalarE ACTIVATE spans a 2-plane PSUM group (N=1022),
halving the ACT instruction count of the dominant phase (ACT is the
bottleneck engine: every silu element costs 1 lane-cycle at 1.2 GHz).

Layout: everything is kept "plane-major" so every matmul moving operand,
every activation input/output, and every copy is contiguous:
  ktp[d, l, w]   = bf16 (k[16w + l, d] + pe[l, d])  -- host provides this
                   fully transposed, so the HBM->SBUF DMA is a straight
                   contiguous copy (no xbar transpose on the device)
  Z group g      = psum [128, 2, 512]: plane e=2g+i from W0e/W0o matmuls
  s{i}[d, p, w]  = silu-merged planes, stage i
The final w_stop matmul uses the data as the stationary operand, producing
output already row-major for a clean single DMA out per head.
"""

import numpy as np

B, H, S, D = 2, 16, 8192, 128
BH = B * H
NCORES = 8
HPC = BH // NCORES  # heads per core = 4
NB = (S - 32) // 16 + 1  # 511 sliding windows
NW = NB + 1  # 512 output rows per head (incl. zero window)

# w_stop output chunking: window ranges per PE (stationary) chunk
QRANGES = [(0, 128), (128, 128), (256, 128), (384, 127)]

_BASS_CACHE = {}


def _build_bass():
    import concourse.bacc as bacc
    import concourse.mybir as mybir
    import concourse.tile as tile
    from bass_rust import add_dep_helper

    f32 = mybir.dt.float32
    bf16 = mybir.dt.bfloat16
    SILU = mybir.ActivationFunctionType.Silu

    nc = bacc.Bacc()
    # k4p[hh, d, 512*l + w] = bf16(k[16w + l, d] + pe[l, d]): the l-planar
    # transposed layout, prepared on the host so the device DMA is contiguous.
    k4p = nc.dram_tensor("k4p", [HPC, D, S], bf16, kind="ExternalInput")
    # wdt[k, i, h, o] = w_down[i][o, 128h + k]: already in SBUF layout
    wdt = nc.dram_tensor("wdt", [128, 5, 2, 128], bf16, kind="ExternalInput")
    dpeT = nc.dram_tensor("dpeT", [1, 8, 128], bf16, kind="ExternalInput")
    ones = nc.dram_tensor("ones", [1, 512], bf16, kind="ExternalInput")
    wst = nc.dram_tensor("wst", [128, 128], bf16, kind="ExternalInput")
    oqs = [
        nc.dram_tensor(f"o{hh}", [513, 128], f32, kind="ExternalOutput")
        for hh in range(HPC)
    ]

    with tile.TileContext(nc) as tc:
        with (
            tc.tile_pool(name="consts", bufs=1) as consts,
            tc.tile_pool(name="ktp", bufs=2) as ktp,
            tc.tile_pool(name="s0p", bufs=2) as s0p,
            tc.tile_pool(name="stp", bufs=2) as stp,
            tc.tile_pool(name="outp", bufs=2) as outp,
            tc.tile_pool(name="tps", bufs=1, space="PSUM") as tps,
            tc.tile_pool(name="zps", bufs=1, space="PSUM") as zps,
            tc.tile_pool(name="sps", bufs=2, space="PSUM") as sps,
            tc.tile_pool(name="ops", bufs=1, space="PSUM") as ops,
        ):
            # constants go on the ACT HWDGE ring (separate queue from the k
            # loads on the SP ring), as plain contiguous copies
            wd_sb = consts.tile([128, 5, 2, 128], bf16, name="wd_sb")
            nc.scalar.dma_start(out=wd_sb, in_=wdt[:])
            wst_sb = consts.tile([128, 128], bf16, name="wst_sb")
            nc.scalar.dma_start(out=wst_sb, in_=wst[:])
            dpeT_sb = consts.tile([1, 8, 128], bf16, name="dpeT_sb")
            nc.scalar.dma_start(out=dpeT_sb, in_=dpeT[:])
            ones_sb = consts.tile([1, 512], bf16, name="ones_sb")
            nc.scalar.dma_start(out=ones_sb, in_=ones[:])

            # The walrus pipeline fuses a matmul's sem waits into its
            # LDWEIGHTS slot (1 wait); extra waits cost an EventSemaphore
            # instruction.  Absorber matmuls make PE observe each semaphore
            # cheaply first.  They write disjoint 2-col regions of one
            # never-read PSUM bank (no WAW sems between them).
            dummy = tps.tile([128, 512], f32, name="dummy", tag="dummy", bufs=1)
            dummy_ctr = [0]

            def absorb(lhsT, rhs):
                m = dummy_ctr[0]
                dummy_ctr[0] += 1
                dst = dummy[: lhsT.shape[-1], 2 * m : 2 * m + 2]
                return nc.tensor.matmul(dst, lhsT=lhsT, rhs=rhs, start=True, stop=True)

            absorb(wd_sb[:, 0, 0, 0:2], wd_sb[:, 0, 0, 0:2])
            absorb(wst_sb[:, 0:2], wst_sb[:, 0:2])
            absorb(dpeT_sb[:, 0, 0:2], ones_sb[:, 0:2])

            def z_units(hh, s0_out):
                """Generator: per-group units of the shared stage-0 (Z) phase.

                Group g covers Z planes e=2g, 2g+1 in one [128, 2, 512] PSUM
                tile (2 banks, single-buffered).  Sequence per group: 4 Z
                matmuls; silu A over cols 0:511 of both planes (s0 planes
                2g, 2g+1); dpe bias outer-product matmuls into cols 1:512;
                silu B over cols 1:512 (s0 planes 2g+8, 2g+9).  Issue order
                keeps the in-order PE queue deadlock-free with the
                single-buffered bank pair: bias(g) strictly before Zmm(g+1).
                Yields after A and after B so the interleaved stage units of
                the previous head backfill both engines' dependency stalls.
                """
                ktf = ktp.tile([128, S], bf16, name="ktf")
                kt3 = ktf.rearrange("p (l w) -> p l w", w=512)
                # 4 chunked contiguous DMAs (4 l-planes each) so the first Z
                # matmuls start as soon as the first chunk lands
                for c in range(4):
                    nc.sync.dma_start(
                        out=ktf[:, 2048 * c : 2048 * (c + 1)],
                        in_=k4p[hh, :, 2048 * c : 2048 * (c + 1)],
                    )
                s0 = s0p.tile([128, 16, NB], bf16, name="s0")
                s0_out[hh] = s0

                def zmm_group(g):
                    # observe this chunk's DMA lane on PE
                    absorb(kt3[:, 4 * g, 0:2], kt3[:, 4 * g, 0:2])
                    zp = zps.tile([128, 2, 512], f32, name="zp", tag="zp")
                    for i in range(2):
                        e = 2 * g + i
                        nc.tensor.matmul(
                            zp[:, i, :], lhsT=wd_sb[:, 0, 0, :],
                            rhs=kt3[:, 2 * e, :], start=True, stop=False,
                        )
                        nc.tensor.matmul(
                            zp[:, i, :], lhsT=wd_sb[:, 0, 1, :],
                            rhs=kt3[:, 2 * e + 1, :], start=False, stop=True,
                        )
                    return zp

                for g in range(4):
                    zp = zmm_group(g)
                    nc.scalar.activation(
                        out=s0[:, 2 * g : 2 * g + 2, :],
                        in_=zp[:, :, 0:NB], func=SILU,
                    )
                    yield
                    for i in range(2):
                        e = 2 * g + i
                        nc.tensor.matmul(
                            zp[:, i, 1:512], lhsT=dpeT_sb[:, e, :],
                            rhs=ones_sb[:, 0:NB], start=False, stop=True,
                            skip_group_check=True,
                        )
                    nc.scalar.activation(
                        out=s0[:, 2 * g + 8 : 2 * g + 10, :],
                        in_=zp[:, :, 1 : NB + 1], func=SILU,
                    )
                    yield

            def stage_units(hh, s0):
                """Generator: per-silu-group units of stages 1..4 + w_stop."""
                prev = s0
                for st in range(1, 5):
                    nj = 16 >> st
                    cur = stp.tile([128, nj, NB], bf16, name=f"s{st}", tag=f"s{st}")
                    for p in range((nj + 1) // 2):
                        npl = min(2, nj - 2 * p)
                        ps = sps.tile([128, 2, 512], f32, name="ps", tag="sp")
                        for ii in range(npl):
                            i = 2 * p + ii
                            nc.tensor.matmul(
                                ps[:, ii, :NB], lhsT=wd_sb[:, st, 0, :],
                                rhs=prev[:, 2 * i, :],
                                start=True, stop=False,
                            )
                            nc.tensor.matmul(
                                ps[:, ii, :NB], lhsT=wd_sb[:, st, 1, :],
                                rhs=prev[:, 2 * i + 1, :],
                                start=False, stop=True,
                            )
                        nc.scalar.activation(
                            out=cur[:, 2 * p : 2 * p + npl, :],
                            in_=ps[:, :npl, :NB], func=SILU,
                        )
                        yield
                    prev = cur

                # w_stop with data-stationary -> row-major [w, o] output;
                # all 4 chunks packed into one PSUM bank, single DVE copy,
                # single DMA out (rows 1..512 of o[hh]; row 512 is garbage
                # from the 127-wide last chunk, sliced off on the host)
                s4f = prev[:, 0, :]  # [128, 511]
                outsb = outp.tile([128, 4, 128], f32, name="outsb")
                ab2 = absorb(s4f[:, 0:2], s4f[:, 0:2])
                ps2 = ops.tile([128, 4, 128], f32, name="ps2", tag="op")
                for q, (w0, wq) in enumerate(QRANGES):
                    mmq = nc.tensor.matmul(
                        ps2[:wq, q, :],
                        lhsT=s4f[:, w0 : w0 + wq],
                        rhs=wst_sb,
                        start=True, stop=True,
                    )
                    if q == 0:
                        add_dep_helper(mmq.ins, ab2.ins, False,
                                       "absorber before first stop matmul")
                nc.vector.tensor_copy(out=outsb, in_=ps2)
                o_view = oqs[hh][1:513].rearrange("(q p) o -> p q o", q=4)
                if hh == HPC - 1:
                    # SP ring is drained of k loads by now; HWDGE is faster
                    nc.sync.dma_start(out=o_view, in_=outsb)
                else:
                    nc.gpsimd.dma_start(out=o_view, in_=outsb)
                yield

            # software pipeline with fine-grained interleave: the stage/stop
            # units of head h alternate with the Z units of head h+1, so the
            # in-order ACT/PE streams always have independent work to backfill
            # dependency gaps
            s0s = {}
            for _ in z_units(0, s0s):
                pass
            for hh in range(HPC):
                gens = [stage_units(hh, s0s.pop(hh))]
                if hh + 1 < HPC:
                    gens.append(z_units(hh + 1, s0s))
                while gens:
                    for g in list(gens):
                        try:
                            next(g)
                        except StopIteration:
                            gens.remove(g)

    if not nc.is_finalized():
        nc.finalize()
    return nc


def _prep_host_inputs(k, pe, w_down, w_stop):
    import ml_dtypes

    bf16 = ml_dtypes.bfloat16
    k = np.asarray(k, dtype=np.float32)
    pe = np.asarray(pe, dtype=np.float32)
    w_down = np.asarray(w_down, dtype=np.float32)
    w_stop = np.asarray(w_stop, dtype=np.float32)

    # k4p[bh, d, 512*l + w] = k[bh, 16w+l, d] + pe[l, d], cast to bf16 (RNE):
    # the fully transposed l-planar layout with the use-A pe pre-added, so the
    # device-side DMA is a straight contiguous copy
    kr = k.reshape(BH, 512, 16, D) + pe[:16][None, None, :, :]
    k4p = np.ascontiguousarray(kr.transpose(0, 3, 2, 1)).astype(bf16)
    # wdt[k, i, h, o] = w_down[i][o, 128h + k]
    wdt = np.ascontiguousarray(
        w_down.reshape(5, 128, 2, 128).transpose(3, 0, 2, 1)
    ).astype(bf16)
    # dpeT[e, o] = (W0 @ (pe_pair(e+8) - pe_pair(e)))[o]: use-B bias correction
    pe_pairs = pe.reshape(16, 256).astype(np.float64)
    dpeT = (
        (pe_pairs[8:] - pe_pairs[:8]) @ w_down[0].astype(np.float64).T
    ).astype(np.float32).astype(bf16).reshape(1, 8, 128)
    ones = np.ones((1, 512), dtype=bf16)
    wst = np.ascontiguousarray(w_stop.T).astype(bf16)
    return k4p, wdt, dpeT, ones, wst


def run(k, pe, w_down, w_stop, trace=False, trace_kwargs=None):
    from concourse.bass_utils import run_bass_kernel_spmd

    k4p, wdt, dpeT, ones, wst = _prep_host_inputs(k, pe, w_down, w_stop)

    if "nc" not in _BASS_CACHE:
        _BASS_CACHE["nc"] = _build_bass()
    nc = _BASS_CACHE["nc"]

    in_maps = [
        {
            "k4p": np.ascontiguousarray(k4p[HPC * c : HPC * (c + 1)]),
            "wdt": wdt,
            "dpeT": dpeT,
            "ones": ones,
            "wst": wst,
        }
        for c in range(NCORES)
    ]
    res = run_bass_kernel_spmd(
        nc, in_maps, core_ids=list(range(NCORES)), trace=trace,
        **(trace_kwargs or {}),
    )
    out = np.empty((BH, NW, D), dtype=np.float32)
    for c in range(NCORES):
        r = res.results[c]
        for hh in range(HPC):
            row = HPC * c + hh
            out[row, 0, :] = 0.0
            out[row, 1:NW, :] = r[f"o{hh}"][1:NW]
    out = out.reshape(B, H, NW, D)
    return out, res


def kernel(k, pe, w_down, w_stop):
    out, _ = run(k, pe, w_down, w_stop, trace=False)
    return out


# revision 11
# speedup vs baseline: 1.3074x; 1.1754x over previous
"""Trainium2 Bass kernel for nn_Compressor (sparse_attention, hierarchical window MLP).

Reference computation (per batch b, head h):
  windows w=0..510 over k[b,h] (S=8192, D=128), window length 32, stride 16
  x[w, l, :] = k[16w+l, :] + pe[l, :]
  5 stages of pairwise-merge MLP: x <- silu(x.reshape(-1, 256) @ w_down[i].T)
  out[w+1] = x @ w_stop.T   ; out[0] = 0 (prepended zero window)

Sharding: head-parallel across 8 cores (B*H = 32 -> 4 heads/core), weights
replicated, no cross-device comms.

Algebraic optimization: stage-0 operates on adjacent row pairs (s=2t, 2t+1)
and every pair is shared by exactly two windows (stride 16, pair width 2),
always in the same even/odd role.  So
  Z[:, t] = W0_even @ kT[:, 2t] + W0_odd @ kT[:, 2t+1]
is computed once per pair (half the naive stage-0 flops).  The host pre-adds
pe[l] (the use-A positional encoding, position l = s mod 16) into k itself,
so the use-A silu needs no bias; the second use of each pair (use B,
position l+16 in the previous window) differs only by the linear image of
the pe difference,
  dpe0 = W0 @ (pe_B - pe_A)  [128, 8],
which a K=1 PE matmul (dpe column x ones-row outer product) accumulates into
the Z PSUM bank between the two silu reads.  Both stage-0 silus are then
bias-free, so each ScalarE ACTIVATE spans a 2-plane PSUM group (N=1022),
halving the ACT instruction count of the dominant phase (ACT is the
bottleneck engine: every silu element costs 1 lane-cycle at 1.2 GHz).

Layout: everything is kept "plane-major" so every matmul moving operand,
every activation input/output, and every copy is contiguous:
  ktp[d, l, w]   = bf16 (k[16w + l, d] + pe[l, d])  -- host provides this
                   fully transposed, so the HBM->SBUF DMA is a straight
                   contiguous copy (no xbar transpose on the device)
  Z group g      = psum [128, 2, 512]: plane e=2g+i from W0e/W0o matmuls
  s{i}[d, p, w]  = silu-merged planes, stage i
The final w_stop matmul uses the data as the stationary operand, producing
output already row-major for a clean single DMA out per head.
"""

import numpy as np

B, H, S, D = 2, 16, 8192, 128
BH = B * H
NCORES = 8
HPC = BH // NCORES  # heads per core = 4
NB = (S - 32) // 16 + 1  # 511 sliding windows
NW = NB + 1  # 512 output rows per head (incl. zero window)

# w_stop output chunking: window ranges per PE (stationary) chunk
QRANGES = [(0, 128), (128, 128), (256, 128), (384, 127)]

_BASS_CACHE = {}


def _build_bass():
    import concourse.bacc as bacc
    import concourse.mybir as mybir
    import concourse.tile as tile
    from bass_rust import add_dep_helper

    f32 = mybir.dt.float32
    bf16 = mybir.dt.bfloat16
    SILU = mybir.ActivationFunctionType.Silu

    nc = bacc.Bacc()
    # k4p[hh, d, 512*l + w] = bf16(k[16w + l, d] + pe[l, d]): the l-planar
    # transposed layout, prepared on the host so the device DMA is contiguous.
    k4p = nc.dram_tensor("k4p", [HPC, D, S], bf16, kind="ExternalInput")
    # wdt[k, i, h, o] = w_down[i][o, 128h + k]: already in SBUF layout
    wdt = nc.dram_tensor("wdt", [128, 5, 2, 128], bf16, kind="ExternalInput")
    dpe = nc.dram_tensor("dpe", [128, 8], f32, kind="ExternalInput")
    wst = nc.dram_tensor("wst", [128, 128], bf16, kind="ExternalInput")
    oqs = [
        nc.dram_tensor(f"o{hh}", [513, 128], f32, kind="ExternalOutput")
        for hh in range(HPC)
    ]

    with tile.TileContext(nc) as tc:
        with (
            tc.tile_pool(name="consts", bufs=1) as consts,
            tc.tile_pool(name="ktp", bufs=2) as ktp,
            tc.tile_pool(name="s0p", bufs=2) as s0p,
            tc.tile_pool(name="stp", bufs=2) as stp,
            tc.tile_pool(name="outp", bufs=2) as outp,
            tc.tile_pool(name="tps", bufs=1, space="PSUM") as tps,
            tc.tile_pool(name="zps", bufs=2, space="PSUM") as zps,
            tc.tile_pool(name="sps", bufs=2, space="PSUM") as sps,
            tc.tile_pool(name="ops", bufs=1, space="PSUM") as ops,
        ):
            # constants go on the ACT HWDGE ring (separate queue from the k
            # loads on the SP ring), as plain contiguous copies
            wd_sb = consts.tile([128, 5, 2, 128], bf16, name="wd_sb")
            nc.scalar.dma_start(out=wd_sb, in_=wdt[:])
            wst_sb = consts.tile([128, 128], bf16, name="wst_sb")
            nc.scalar.dma_start(out=wst_sb, in_=wst[:])
            dpe_sb = consts.tile([128, 8], f32, name="dpe_sb")
            nc.scalar.dma_start(out=dpe_sb, in_=dpe[:])

            # The walrus pipeline fuses a matmul's sem waits into its
            # LDWEIGHTS slot (1 wait); extra waits cost an EventSemaphore
            # instruction.  Absorber matmuls make PE observe each semaphore
            # cheaply first.  They write disjoint 2-col regions of one
            # never-read PSUM bank (no WAW sems between them).
            dummy = tps.tile([128, 512], f32, name="dummy", tag="dummy", bufs=1)
            dummy_ctr = [0]

            def absorb(lhsT, rhs):
                m = dummy_ctr[0]
                dummy_ctr[0] += 1
                dst = dummy[: lhsT.shape[-1], 2 * m : 2 * m + 2]
                return nc.tensor.matmul(dst, lhsT=lhsT, rhs=rhs, start=True, stop=True)

            absorb(wd_sb[:, 0, 0, 0:2], wd_sb[:, 0, 0, 0:2])
            absorb(wst_sb[:, 0:2], wst_sb[:, 0:2])

            def z_units(hh, s0_out):
                """Generator: per-group units of the shared stage-0 (Z) phase.

                Plane e's Z lands in one [128, 512] PSUM bank; silu A over
                cols 0:511 gives s0 plane e (bias-free, pe pre-added on the
                host); silu B over cols 1:512 with the dpe bias argument
                gives s0 plane e+8.
                """
                ktf = ktp.tile([128, S], bf16, name="ktf")
                kt3 = ktf.rearrange("p (l w) -> p l w", w=512)
                # 4 chunked contiguous DMAs (4 l-planes each) so the first Z
                # matmuls start as soon as the first chunk lands
                for c in range(4):
                    nc.sync.dma_start(
                        out=ktf[:, 2048 * c : 2048 * (c + 1)],
                        in_=k4p[hh, :, 2048 * c : 2048 * (c + 1)],
                    )
                s0 = s0p.tile([128, 16, NB], bf16, name="s0")
                s0_out[hh] = s0
                for e in range(8):
                    if e % 2 == 0:
                        # observe this chunk's DMA lane on PE
                        absorb(kt3[:, 2 * e, 0:2], kt3[:, 2 * e, 0:2])
                    zp = zps.tile([128, 512], f32, name="zp", tag="zp")
                    nc.tensor.matmul(
                        zp, lhsT=wd_sb[:, 0, 0, :], rhs=kt3[:, 2 * e, :],
                        start=True, stop=False,
                    )
                    nc.tensor.matmul(
                        zp, lhsT=wd_sb[:, 0, 1, :], rhs=kt3[:, 2 * e + 1, :],
                        start=False, stop=True,
                    )
                    nc.scalar.activation(
                        out=s0[:, e, :], in_=zp[:, 0:NB], func=SILU,
                    )
                    nc.scalar.activation(
                        out=s0[:, e + 8, :], in_=zp[:, 1 : NB + 1], func=SILU,
                        bias=dpe_sb[:, e : e + 1], scale=1.0,
                    )
                    yield

            def stage_units(hh, s0):
                """Generator: per-silu-group units of stages 1..4 + w_stop."""
                prev = s0
                for st in range(1, 5):
                    nj = 16 >> st
                    cur = stp.tile([128, nj, NB], bf16, name=f"s{st}", tag=f"s{st}")
                    for p in range((nj + 1) // 2):
                        npl = min(2, nj - 2 * p)
                        ps = sps.tile([128, 2, 512], f32, name="ps", tag="sp")
                        for ii in range(npl):
                            i = 2 * p + ii
                            nc.tensor.matmul(
                                ps[:, ii, :NB], lhsT=wd_sb[:, st, 0, :],
                                rhs=prev[:, 2 * i, :],
                                start=True, stop=False,
                            )
                            nc.tensor.matmul(
                                ps[:, ii, :NB], lhsT=wd_sb[:, st, 1, :],
                                rhs=prev[:, 2 * i + 1, :],
                                start=False, stop=True,
                            )
                        nc.scalar.activation(
                            out=cur[:, 2 * p : 2 * p + npl, :],
                            in_=ps[:, :npl, :NB], func=SILU,
                        )
                        yield
                    prev = cur

                # w_stop with data-stationary -> row-major [w, o] output;
                # all 4 chunks packed into one PSUM bank, single DVE copy,
                # single DMA out (rows 1..512 of o[hh]; row 512 is garbage
                # from the 127-wide last chunk, sliced off on the host)
                s4f = prev[:, 0, :]  # [128, 511]
                outsb = outp.tile([128, 4, 128], f32, name="outsb")
                ab2 = absorb(s4f[:, 0:2], s4f[:, 0:2])
                ps2 = ops.tile([128, 4, 128], f32, name="ps2", tag="op")
                for q, (w0, wq) in enumerate(QRANGES):
                    mmq = nc.tensor.matmul(
                        ps2[:wq, q, :],
                        lhsT=s4f[:, w0 : w0 + wq],
                        rhs=wst_sb,
                        start=True, stop=True,
                    )
                    if q == 0:
                        add_dep_helper(mmq.ins, ab2.ins, False,
                                       "absorber before first stop matmul")
                nc.vector.tensor_copy(out=outsb, in_=ps2)
                o_view = oqs[hh][1:513].rearrange("(q p) o -> p q o", q=4)
                if hh == HPC - 1:
                    # SP ring is drained of k loads by now; HWDGE is faster
                    nc.sync.dma_start(out=o_view, in_=outsb)
                else:
                    nc.gpsimd.dma_start(out=o_view, in_=outsb)
                yield

            # software pipeline with fine-grained interleave: the stage/stop
            # units of head h alternate with the Z units of head h+1, so the
            # in-order ACT/PE streams always have independent work to backfill
            # dependency gaps
            s0s = {}
            for _ in z_units(0, s0s):
                pass
            for hh in range(HPC):
                gens = [stage_units(hh, s0s.pop(hh))]
                if hh + 1 < HPC:
                    gens.append(z_units(hh + 1, s0s))
                while gens:
                    for g in list(gens):
                        try:
                            next(g)
                        except StopIteration:
                            gens.remove(g)

    if not nc.is_finalized():
        nc.finalize()
    return nc


def _prep_host_inputs(k, pe, w_down, w_stop):
    import ml_dtypes

    bf16 = ml_dtypes.bfloat16
    k = np.asarray(k, dtype=np.float32)
    pe = np.asarray(pe, dtype=np.float32)
    w_down = np.asarray(w_down, dtype=np.float32)
    w_stop = np.asarray(w_stop, dtype=np.float32)

    # k4p[bh, d, 512*l + w] = k[bh, 16w+l, d] + pe[l, d], cast to bf16 (RNE):
    # the fully transposed l-planar layout with the use-A pe pre-added, so the
    # device-side DMA is a straight contiguous copy
    kr = k.reshape(BH, 512, 16, D) + pe[:16][None, None, :, :]
    k4p = np.ascontiguousarray(kr.transpose(0, 3, 2, 1)).astype(bf16)
    # wdt[k, i, h, o] = w_down[i][o, 128h + k]
    wdt = np.ascontiguousarray(
        w_down.reshape(5, 128, 2, 128).transpose(3, 0, 2, 1)
    ).astype(bf16)
    # dpe[o, e] = (W0 @ (pe_pair(e+8) - pe_pair(e)))[o]: use-B bias correction
    pe_pairs = pe.reshape(16, 256).astype(np.float64)
    dpe = (
        w_down[0].astype(np.float64) @ (pe_pairs[8:] - pe_pairs[:8]).T
    ).astype(np.float32)
    wst = np.ascontiguousarray(w_stop.T).astype(bf16)
    return k4p, wdt, dpe, wst


def run(k, pe, w_down, w_stop, trace=False, trace_kwargs=None):
    from concourse.bass_utils import run_bass_kernel_spmd

    k4p, wdt, dpe, wst = _prep_host_inputs(k, pe, w_down, w_stop)

    if "nc" not in _BASS_CACHE:
        _BASS_CACHE["nc"] = _build_bass()
    nc = _BASS_CACHE["nc"]

    in_maps = [
        {
            "k4p": np.ascontiguousarray(k4p[HPC * c : HPC * (c + 1)]),
            "wdt": wdt,
            "dpe": dpe,
            "wst": wst,
        }
        for c in range(NCORES)
    ]
    res = run_bass_kernel_spmd(
        nc, in_maps, core_ids=list(range(NCORES)), trace=trace,
        **(trace_kwargs or {}),
    )
    out = np.empty((BH, NW, D), dtype=np.float32)
    for c in range(NCORES):
        r = res.results[c]
        for hh in range(HPC):
            row = HPC * c + hh
            out[row, 0, :] = 0.0
            out[row, 1:NW, :] = r[f"o{hh}"][1:NW]
    out = out.reshape(B, H, NW, D)
    return out, res


def kernel(k, pe, w_down, w_stop):
    out, _ = run(k, pe, w_down, w_stop, trace=False)
    return out
